# revision 33
# baseline (speedup 1.0000x reference)
"""Trainium2 Bass kernel for a masked-attention block (MAB).

Computation (per batch element):
    Q = X@Wq + bq ; K = Y@Wk + bk ; V = Y@Wv + bv
    logits = per-head Qh@Kh^T / 32, masked keys -> -inf, softmax over keys
    attn   = A @ Vh (concat heads)
    O1 = LN(Q + attn; g1,b1)
    O  = LN(O1 + relu(O1@Wo + bo); g2,b2)

Sharding: pure data-parallel, one batch element per NeuronCore (B=8 = 8 cores).

On-device dataflow is "feature-major": activations live in SBUF transposed
([model_dim -> 8x128 partitions, token -> free]).  With weights in natural
layout every matmul chains without any transposes.  All matmul operands are
bf16 (PE rate is identical to fp32r, but: half the DMA bytes, FWL-accelerated
LDWEIGHTS, and 2x packed DVE ops); PSUM accumulation stays fp32.

Schedule (engines run their queues in program order; this ordering is the
software pipeline):
  pre-loop : V proj (natural layout), Q proj        [PE; ACT does epilogues]
  loop h   : denom tree for h-1 [DVE], K proj h [PE, DVE epi], logits h [PE],
             exp h [ACT], AV h-1 [PE], attn epilogue h-1 [DVE]
  tail     : LN1 -> O proj -> LN2 per 512-token half, pipelined across
             PE (stats matmuls, proj) / DVE (elementwise) / ACT (affine)

The softmax denominator is a partition-dim reduction done as a bf16 pairwise
tree on DVE (frees the PE of ~65k ones-matmul columns); LayerNorm stats stay
as all-ones stationary matmuls (cheap, and they broadcast for free).

The host transposes X/Y on the way in and the output on the way out, converts
everything the matmuls touch to bf16, and turns the bool mask into an
additive f32 bias (0 / -1e4) consumed by the exp activation.
"""

import math
import numpy as np
from contextlib import ExitStack

import ml_dtypes

import concourse.bass as bass
import concourse.mybir as mybir
import concourse.tile as tile
from concourse import bacc
from concourse.bass_utils import run_bass_kernel_spmd

P = 128
NX = 1024
NY = 1024
DIM = 1024
H = 8
KO = DIM // P          # 8 partition sub-tiles of the model dim
QC = 512               # moving-operand chunk
NQC = NX // QC         # 2
F32 = mybir.dt.float32
BF16 = mybir.dt.bfloat16
AF = mybir.ActivationFunctionType
ALU = mybir.AluOpType
SCALE = 1.0 / 32.0     # 1/sqrt(DIM)
EPS = 1e-5
DEBUG = False          # adds intermediate-tensor DRAM dumps (debugging only)


def _build():
    nc = bacc.Bacc("TRN2", target_bir_lowering=False, debug=False,
                   enable_asserts=False)

    # ---- DRAM I/O (per-core shapes) ----
    XT = nc.dram_tensor("XT", [DIM, NX], BF16, kind="ExternalInput").ap()
    YT = nc.dram_tensor("YT", [DIM, NY], BF16, kind="ExternalInput").ap()
    MB = nc.dram_tensor("MB", [NY], F32, kind="ExternalInput").ap()
    Wd = {}
    for w in ("Wq", "Wk", "Wv", "Wo"):
        Wd[w] = nc.dram_tensor(w, [DIM, DIM], BF16, kind="ExternalInput").ap()
    Vecs = {}
    for vname in ("bq", "bk", "bv", "bo", "g1", "b1", "g2", "b2"):
        Vecs[vname] = nc.dram_tensor(vname, [DIM], F32, kind="ExternalInput").ap()
    OT = nc.dram_tensor("OT", [DIM, NX], BF16, kind="ExternalOutput").ap()

    xt3 = XT.rearrange("(ko p) q -> p ko q", p=P)
    yt3 = YT.rearrange("(ko p) q -> p ko q", p=P)
    wq3 = Wd["Wq"].rearrange("(ko p) d -> p ko d", p=P)
    wk3 = Wd["Wk"].rearrange("(ko p) d -> p ko d", p=P)
    wv3 = Wd["Wv"].rearrange("(ko p) d -> p ko d", p=P)
    wo3 = Wd["Wo"].rearrange("(ko p) d -> p ko d", p=P)
    ot3 = OT.rearrange("(do p) q -> p do q", p=P)

    dbg = {}
    if DEBUG:
        for nm, shp, dt in [("d_qt", [P, KO, NX], BF16),
                            ("d_ktm0", [P, NY], BF16),
                            ("d_vm", [P, KO, DIM], BF16),
                            ("d_et00", [P, NX], BF16),
                            ("d_rc0", [P, NX], F32),
                            ("d_zt", [P, KO, NX], BF16),
                            ("d_o1t", [P, KO, NX], BF16),
                            ("d_z2t", [P, KO, NX], BF16)]:
            dbg[nm] = nc.dram_tensor(nm, shp, dt, kind="ExternalOutput").ap()

    with tile.TileContext(nc) as tc:
        with ExitStack() as octx:
            const = octx.enter_context(tc.tile_pool(name="const", bufs=1))
            persist = octx.enter_context(tc.tile_pool(name="persist", bufs=1))
            actp = octx.enter_context(tc.tile_pool(name="act", bufs=3))

            # ---- constants (issue the small DMAs first on the sync queue) ----
            ones_bf = const.tile([P, P], BF16, tag="onesbf", name="ones_bf")
            nc.vector.memset(ones_bf, 1.0)
            eps_sb = const.tile([P, 1], F32, tag="eps", name="eps_sb")
            nc.vector.memset(eps_sb, EPS)

            # vector constants: tiles now, DMAs issued after the big input
            # tensors (nothing reads them before ~45us)
            def vec_tile(name):
                return const.tile([P, KO], F32, tag=f"v_{name}", name=f"{name}_sb")

            vec_names = ("bq", "bk", "bv", "bo", "g1", "b1", "g2", "b2")
            vec_sb = {n: vec_tile(n) for n in vec_names}
            mb_sb = const.tile([P, KO], F32, tag="v_mb", name="mb_sb")
            bq_sb, bk_sb, bv_sb, bo_sb = (vec_sb[n] for n in vec_names[:4])
            g1_sb, b1_sb, g2_sb, b2_sb = (vec_sb[n] for n in vec_names[4:])

            def issue_vec_dmas():
                nc.sync.dma_start(mb_sb, MB.rearrange("(ko p) -> p ko", p=P))
                for n in vec_names:
                    nc.sync.dma_start(
                        vec_sb[n], Vecs[n].rearrange("(ko p) -> p ko", p=P))

            # ---- persistent tensors ----
            yt = persist.tile([P, KO, NY], BF16, tag="yt", name="yt")
            vm = persist.tile([P, KO, DIM], BF16, tag="vm", name="vm")
            wo_sb = persist.tile([P, KO, DIM], BF16, tag="wo", name="wo_sb")
            wkp = octx.enter_context(tc.tile_pool(name="wkp", bufs=2))
            ktmp = octx.enter_context(tc.tile_pool(name="ktmp", bufs=2))

            # big feature-major activation tiles (rotate through 3 slots)
            qt = actp.tile([P, KO, NX], BF16, tag="big", name="qt")

            # ================= Phase 1: V and Q projections =================
            with tc.tile_pool(name="io", bufs=1) as iop, \
                 tc.tile_pool(name="gp1", bufs=8, space="PSUM") as pp:
                wv_sb = iop.tile([P, KO, DIM], BF16, tag="wv", name="wv_sb")
                xt = iop.tile([P, KO, NX], BF16, tag="xt", name="xt")
                wq_sb = iop.tile([P, KO, DIM], BF16, tag="wq", name="wq_sb")
                # per-k chunked DMAs, interleaved by priority so the V proj
                # can start as soon as the first (yt, wv) chunk pair lands
                for k in range(KO):
                    nc.sync.dma_start(yt[:, k, :], yt3[:, k, :])
                    nc.sync.dma_start(wv_sb[:, k, :], wv3[:, k, :])
                for k in range(KO):
                    nc.sync.dma_start(xt[:, k, :], xt3[:, k, :])
                    nc.sync.dma_start(wq_sb[:, k, :], wq3[:, k, :])
                wk_tiles = []
                for h in range(2):
                    wkt = wkp.tile([P, KO, P], BF16, tag="wk", name=f"wk{h}")
                    nc.sync.dma_start(wkt, wk3[:, :, h * P:(h + 1) * P])
                    wk_tiles.append(wkt)
                issue_vec_dmas()
                nc.sync.dma_start(wo_sb, wo3)

                # V in natural (token-major) layout: V[y, n] = sum_k Y[y,k] Wv[k,n]
                # (bias bv is NOT added here: softmax rows sum to 1, so it is
                # folded into the attention epilogue instead)
                for yo in range(KO):
                    pss = [pp.tile([P, QC], F32, tag="ps", name=f"ps_v{yo}{ng}")
                           for ng in range(2)]
                    for k in range(KO):
                        for ng in range(2):
                            ns = slice(ng * QC, (ng + 1) * QC)
                            nc.tensor.matmul(
                                pss[ng],
                                lhsT=yt[:, k, yo * P:(yo + 1) * P],
                                rhs=wv_sb[:, k, ns],
                                start=(k == 0), stop=(k == KO - 1))
                    for ng in range(2):
                        ns = slice(ng * QC, (ng + 1) * QC)
                        nc.scalar.copy(vm[:, yo, ns], pss[ng])

                # Q feature-major: qt[p, do, q] = sum_k Wq[k, d] xt[k, q] + bq
                for do in range(KO):
                    for qc in range(NQC):
                        qs = slice(qc * QC, (qc + 1) * QC)
                        ps = pp.tile([P, QC], F32, tag="ps", name=f"ps_q{do}{qc}")
                        for k in range(KO):
                            nc.tensor.matmul(
                                ps,
                                lhsT=wq_sb[:, k, do * P:(do + 1) * P],
                                rhs=xt[:, k, qs],
                                start=(k == 0), stop=(k == KO - 1))
                        nc.scalar.activation(
                            qt[:, do, qs], ps, AF.Identity,
                            bias=bq_sb[:, do:do + 1], scale=1.0)

            if DEBUG:
                nc.sync.dma_start(dbg["d_qt"], qt)
                nc.sync.dma_start(dbg["d_vm"], vm)

            # ================= Phase 2: K proj + attention (pipelined) ======
            zt = actp.tile([P, KO, NX], BF16, tag="big", name="zt")

            with tc.tile_pool(name="kq", bufs=2, space="PSUM") as kqp, \
                 tc.tile_pool(name="lgp", bufs=2, space="PSUM") as lgp, \
                 tc.tile_pool(name="avp", bufs=1, space="PSUM") as avp, \
                 tc.tile_pool(name="exp", bufs=17) as ep, \
                 tc.tile_pool(name="prs", bufs=7) as prp, \
                 tc.tile_pool(name="den", bufs=2) as dnp, \
                 tc.tile_pool(name="rcp", bufs=2) as rcp:

                def kproj(h):
                    # K slab h: ktm_h[p, y] = sum_k Wk[k, h*128+p] yt[k, y] + bk
                    ktm_h = ktmp.tile([P, NY], BF16, tag="ktm", name=f"ktm{h}")
                    for qc in range(NQC):
                        qs = slice(qc * QC, (qc + 1) * QC)
                        pk = kqp.tile([P, QC], F32, tag="pk", name=f"pk{h}{qc}")
                        for k in range(KO):
                            nc.tensor.matmul(
                                pk, lhsT=wk_tiles[h][:, k, :], rhs=yt[:, k, qs],
                                start=(k == 0), stop=(k == KO - 1))
                        nc.scalar.activation(
                            ktm_h[:, qs], pk, AF.Identity,
                            bias=bk_sb[:, h:h + 1], scale=1.0)
                    if h + 2 < H:
                        # prefetch the h+2 weight chunk; issued after this
                        # head's matmuls so the 2-deep pool rotation can't
                        # clobber a chunk that still has unissued readers
                        wkt = wkp.tile([P, KO, P], BF16, tag="wk", name=f"wk{h+2}")
                        nc.sync.dma_start(wkt, wk3[:, :, (h + 2) * P:(h + 3) * P])
                        wk_tiles.append(wkt)
                    return ktm_h

                def logits_exp(h, ktm_h):
                    # logitsT[k, q] = sum_d KT_h[d, k] QT_h[d, q]; exp with
                    # mask bias per key (partition) and 1/32 scale
                    et = [ep.tile([P, NX], BF16, tag="exp", name=f"et{h}_{k}")
                          for k in range(KO)]
                    for kt in range(KO):
                        pl = lgp.tile([P, NX], F32, tag="lg", name=f"pl{h}{kt}")
                        for qc in range(NQC):
                            qs = slice(qc * QC, (qc + 1) * QC)
                            nc.tensor.matmul(
                                pl[:, qs],
                                lhsT=ktm_h[:, kt * P:(kt + 1) * P],
                                rhs=qt[:, h, qs],
                                start=True, stop=True)
                        nc.scalar.activation(
                            et[kt], pl, AF.Exp,
                            bias=mb_sb[:, kt:kt + 1], scale=SCALE)
                    return et

                def denom_tree(h, et):
                    # softmax denominator, part 1 (DVE only): cross-tile
                    # pairwise tree (bf16 2x) collapses the 8 key-slab tiles
                    # into one; issued at iteration start to fill the DVE
                    # bubble while the PE runs this head's K projection
                    l1 = [prp.tile([P, NX], BF16, tag="pr", name=f"l1_{h}{i}")
                          for i in range(4)]
                    for i in range(4):
                        # split the first tree level between DVE and the
                        # otherwise-idle GpSimd engine
                        eng = nc.vector if i < 2 else nc.gpsimd
                        eng.tensor_add(l1[i], et[2 * i], et[2 * i + 1])
                    l2 = [prp.tile([P, NX], BF16, tag="pr", name=f"l2_{h}{i}")
                          for i in range(2)]
                    for i in range(2):
                        nc.vector.tensor_add(l2[i], l1[2 * i], l1[2 * i + 1])
                    den = dnp.tile([P, NX], BF16, tag="dn", name=f"den{h}")
                    nc.vector.tensor_add(den, l2[0], l2[1])
                    return den

                def denom_finish(h, den):
                    # part 2: a small all-ones matmul does the in-slab
                    # 128-partition reduction (and broadcasts for free)
                    rc = rcp.tile([P, NX], F32, tag="rc", name=f"rc{h}")
                    for qc in range(NQC):
                        qs = slice(qc * QC, (qc + 1) * QC)
                        pr = kqp.tile([P, QC], F32, tag="pk", name=f"pr{h}{qc}")
                        nc.tensor.matmul(pr, lhsT=ones_bf, rhs=den[:, qs],
                                         start=True, stop=True)
                        nc.vector.reciprocal_approx_fast(rc[:, qs], pr)
                    return rc

                def av_epi(h, et, rc):
                    # attnT_h[d, q] = sum_k V[k, d_h] expT[k, q]; normalize by
                    # the softmax denominator, add bv (softmax rows sum to 1,
                    # so + bv after normalize == bias inside the V proj) and
                    # the Q residual
                    pa = avp.tile([P, NX], F32, tag="av", name=f"pa{h}")
                    for kt in range(KO):
                        for qc in range(NQC):
                            qs = slice(qc * QC, (qc + 1) * QC)
                            nc.tensor.matmul(
                                pa[:, qs],
                                lhsT=vm[:, kt, h * P:(h + 1) * P],
                                rhs=et[kt][:, qs],
                                start=(kt == 0), stop=(kt == KO - 1))
                    # last head: per-qc epilogue so LN1's first chunk can
                    # start a bit earlier
                    for qs in ([slice(0, QC), slice(QC, NX)]
                               if h == H - 1 else [slice(0, NX)]):
                        nc.vector.tensor_mul(zt[:, h, qs], pa[:, qs], rc[:, qs])
                        nc.vector.scalar_tensor_tensor(
                            zt[:, h, qs], zt[:, h, qs], bv_sb[:, h:h + 1],
                            qt[:, h, qs], op0=ALU.add, op1=ALU.add)

                # software pipeline: head h's K proj / logits / exp overlap
                # head h-1's denominator (DVE tree + tiny ones-matmul) and
                # AV (PE).  Issue order is chosen per engine queue: the DVE
                # tree first (ready at iteration start), the PE denominator
                # matmuls after the logits matmuls (so the in-order PE queue
                # never waits on the tree).
                prev = None
                den_prev = None
                for h in range(H):
                    if prev is not None:
                        den_prev = denom_tree(h - 1, prev)
                    ktm_h = kproj(h)
                    et = logits_exp(h, ktm_h)
                    if DEBUG and h == 0:
                        nc.sync.dma_start(dbg["d_ktm0"], ktm_h)
                        nc.sync.dma_start(dbg["d_et00"], et[0])
                    if prev is not None:
                        rc_prev = denom_finish(h - 1, den_prev)
                        if DEBUG and h == 1:
                            nc.sync.dma_start(dbg["d_rc0"], rc_prev)
                        av_epi(h - 1, prev, rc_prev)
                    prev = et
                den_prev = denom_tree(H - 1, prev)
                rc_prev = denom_finish(H - 1, den_prev)
                av_epi(H - 1, prev, rc_prev)
                if DEBUG:
                    nc.sync.dma_start(dbg["d_zt"], zt)

            # ================= Phase 3: LN1 -> O proj + LN2 =================
            # LayerNorm over the model dim (partition direction): stats via
            # all-ones stationary matmuls (free broadcast), elementwise on
            # DVE, gamma/beta affine on ACT.
            with tc.tile_pool(name="sqp", bufs=2) as sqp, \
                 tc.tile_pool(name="stp", bufs=2) as stp, \
                 tc.tile_pool(name="out", bufs=2) as outp, \
                 tc.tile_pool(name="spp", bufs=2, space="PSUM") as spp, \
                 tc.tile_pool(name="gp3", bufs=4, space="PSUM") as pp3:

                def layernorm_chunk(in_sb, lo, hi, g_sb, b_sb, emit_out, lbl):
                    cs = hi - lo
                    qs = slice(lo, hi)
                    pmu = spp.tile([P, QC], F32, tag="pmu", name=f"pmu{lbl}{lo}")
                    for do in range(KO):
                        nc.tensor.matmul(pmu[:, :cs], lhsT=ones_bf,
                                         rhs=in_sb[:, do, qs],
                                         start=(do == 0), stop=(do == KO - 1))
                    sq = sqp.tile([P, KO, QC], BF16, tag="sq", name=f"sq{lbl}{lo}")
                    nc.scalar.square(sq[:, :, :cs], in_sb[:, :, qs])
                    ps2 = spp.tile([P, QC], F32, tag="ps2", name=f"ps2{lbl}{lo}")
                    for do in range(KO):
                        nc.tensor.matmul(ps2[:, :cs], lhsT=ones_bf,
                                         rhs=sq[:, do, :cs],
                                         start=(do == 0), stop=(do == KO - 1))
                    mu = stp.tile([P, QC], F32, tag="mu", name=f"mu{lbl}{lo}")[:, :cs]
                    nc.vector.tensor_scalar_mul(mu, pmu[:, :cs], 1.0 / DIM)
                    msq = stp.tile([P, QC], F32, tag="msq", name=f"msq{lbl}{lo}")[:, :cs]
                    nc.vector.tensor_mul(msq, mu, mu)
                    sd = stp.tile([P, QC], F32, tag="sd", name=f"sd{lbl}{lo}")[:, :cs]
                    nc.vector.scalar_tensor_tensor(
                        sd, ps2[:, :cs], 1.0 / DIM, msq,
                        op0=ALU.mult, op1=ALU.subtract)
                    nc.scalar.activation(sd, sd, AF.Sqrt, bias=eps_sb, scale=1.0)
                    rsig = stp.tile([P, QC], F32, tag="rsig", name=f"rsig{lbl}{lo}")[:, :cs]
                    nc.vector.reciprocal_approx_fast(rsig, sd)
                    mub = stp.tile([P, QC], BF16, tag="mub", name=f"mub{lbl}{lo}")[:, :cs]
                    nc.vector.tensor_copy(mub, mu)
                    rsb = stp.tile([P, QC], BF16, tag="rsb", name=f"rsb{lbl}{lo}")[:, :cs]
                    nc.vector.tensor_copy(rsb, rsig)
                    # t = (x - mu) * rsig over the whole [128, 8, cs] block
                    t = sqp.tile([P, KO, QC], BF16, tag="t", name=f"t{lbl}{lo}")
                    mu_b = mub.unsqueeze(1).broadcast_to([P, KO, cs])
                    rs_b = rsb.unsqueeze(1).broadcast_to([P, KO, cs])
                    nc.vector.tensor_sub(t[:, :, :cs], in_sb[:, :, qs], mu_b)
                    nc.vector.tensor_mul(t[:, :, :cs], t[:, :, :cs], rs_b)
                    for do in range(KO):
                        # out = t * g + b  (per-partition affine, DVE TS 4x)
                        emit_out(do, qs, t[:, do, :cs],
                                 g_sb[:, do:do + 1], b_sb[:, do:do + 1])

                o1t = actp.tile([P, KO, NX], BF16, tag="big", name="o1t")

                def emit_o1(do, qs, t, g_col, b_col):
                    # on ACT: it is idle right here while DVE is the pacer
                    nc.scalar.activation(o1t[:, do, qs], t, AF.Identity,
                                         bias=b_col, scale=g_col)

                z2t = actp.tile([P, KO, NX], BF16, tag="big", name="z2t")

                def oproj_qc(qc):
                    # HT[n, q] = sum_d Wo[d, n] O1T[d, q]; z2 = o1+relu(H+bo)
                    # (relu into a staging tile so the residual add is one
                    # merged DVE op per 512-token half)
                    qs = slice(qc * QC, (qc + 1) * QC)
                    ht = outp.tile([P, KO, QC], BF16, tag="ht", name=f"ht{qc}")
                    for no in range(KO):
                        po = pp3.tile([P, QC], F32, tag="po", name=f"po{no}{qc}")
                        for k in range(KO):
                            nc.tensor.matmul(
                                po,
                                lhsT=wo_sb[:, k, no * P:(no + 1) * P],
                                rhs=o1t[:, k, qs],
                                start=(k == 0), stop=(k == KO - 1))
                        nc.scalar.activation(ht[:, no, :], po, AF.Relu,
                                             bias=bo_sb[:, no:no + 1], scale=1.0)
                        if no == 3:   # first-half residual add starts mid-relu
                            nc.vector.tensor_add(z2t[:, 0:4, qs], ht[:, 0:4, :],
                                                 o1t[:, 0:4, qs])
                    nc.vector.tensor_add(z2t[:, 4:KO, qs], ht[:, 4:KO, :],
                                         o1t[:, 4:KO, qs])

                def make_emit_o2(lo, hi):
                    # affine writes into one staging tile; a single DMA per
                    # chunk replaces 8 per-do DMAs (issue + sem overhead)
                    cs = hi - lo
                    ost = outp.tile([P, KO, QC], BF16, tag="ost",
                                    name=f"ost{lo}")

                    def emit(do, qs, t, g_col, b_col):
                        nc.vector.tensor_scalar(ost[:, do, :cs], t, g_col,
                                                b_col, op0=ALU.mult, op1=ALU.add)
                        if do == KO - 1:
                            nc.sync.dma_start(ot3[:, :, lo:hi], ost[:, :, :cs])
                    return emit

                for qc in range(NQC):
                    layernorm_chunk(zt, qc * QC, (qc + 1) * QC,
                                    g1_sb, b1_sb, emit_o1, "a")
                if DEBUG:
                    nc.sync.dma_start(dbg["d_o1t"], o1t)
                # interleave the second O-proj half behind LN2's first chunk
                # so LN2's stats matmuls don't queue behind all of O-proj on
                # the in-order PE queue; LN2's shrinking chunks keep the
                # serial drain at the very end short
                oproj_qc(0)
                layernorm_chunk(z2t, 0, 512, g2_sb, b2_sb,
                                make_emit_o2(0, 512), "b")
                oproj_qc(1)
                if DEBUG:
                    nc.sync.dma_start(dbg["d_z2t"], z2t)
                for lo, hi in ((512, 768), (768, 1024)):
                    layernorm_chunk(z2t, lo, hi, g2_sb, b2_sb,
                                    make_emit_o2(lo, hi), "b")

    nc.compile()
    return nc


_CACHE = {}


def _get_nc():
    if "nc" not in _CACHE:
        _CACHE["nc"] = _build()
    return _CACHE["nc"]


def make_in_maps(X, Y, mask, Wq, bq, Wk, bk, Wv, bv, Wo, bo, g1, b1, g2, b2):
    bf = lambda a: np.ascontiguousarray(
        np.asarray(a, dtype=np.float32).astype(ml_dtypes.bfloat16))
    f = lambda a: np.ascontiguousarray(np.asarray(a, dtype=np.float32))
    shared = {
        "Wq": bf(Wq), "Wk": bf(Wk), "Wv": bf(Wv), "Wo": bf(Wo),
        "bq": f(bq), "bk": f(bk), "bv": f(bv), "bo": f(bo),
        "g1": f(g1), "b1": f(b1), "g2": f(g2), "b2": f(b2),
    }
    X = np.asarray(X, dtype=np.float32)
    Y = np.asarray(Y, dtype=np.float32)
    mask = np.asarray(mask)
    in_maps = []
    for b in range(8):
        mb = np.where(mask[b], np.float32(-1e4), np.float32(0.0)).astype(np.float32)
        in_maps.append({
            "XT": bf(X[b].T),
            "YT": bf(Y[b].T),
            "MB": mb,
            **shared,
        })
    return in_maps


def kernel(X, Y, mask, Wq, bq, Wk, bk, Wv, bv, Wo, bo, g1, b1, g2, b2,
           _trace=False):
    nc = _get_nc()
    in_maps = make_in_maps(X, Y, mask, Wq, bq, Wk, bk, Wv, bv, Wo, bo,
                           g1, b1, g2, b2)
    res = run_bass_kernel_spmd(nc, in_maps, core_ids=list(range(8)),
                               trace=_trace)
    out = np.stack([np.asarray(res.results[b]["OT"]).astype(np.float32).T
                    for b in range(8)])
    out = np.ascontiguousarray(out)
    if _trace:
        return out, res
    return out


# revision 34
# speedup vs baseline: 1.0829x; 1.0829x over previous
"""Trainium2 Bass kernel for a masked-attention block (MAB).

Computation (per batch element):
    Q = X@Wq + bq ; K = Y@Wk + bk ; V = Y@Wv + bv
    logits = per-head Qh@Kh^T / 32, masked keys -> -inf, softmax over keys
    attn   = A @ Vh (concat heads)
    O1 = LN(Q + attn; g1,b1)
    O  = LN(O1 + relu(O1@Wo + bo); g2,b2)

Sharding: pure data-parallel, one batch element per NeuronCore (B=8 = 8 cores).

On-device dataflow is "feature-major": activations live in SBUF transposed
([model_dim -> 8x128 partitions, token -> free]).  With weights in natural
layout every matmul chains without any transposes.  All matmul operands are
bf16 (PE rate is identical to fp32r, but: half the DMA bytes, FWL-accelerated
LDWEIGHTS, and 2x packed DVE ops); PSUM accumulation stays fp32.

Schedule (engines run their queues in program order; this ordering is the
software pipeline):
  pre-loop : V proj (natural layout), Q proj        [PE; ACT does epilogues]
  loop h   : denom tree for h-1 [DVE], K proj h [PE, DVE epi], logits h [PE],
             exp h [ACT], AV h-1 [PE], attn epilogue h-1 [DVE]
  tail     : LN1 -> O proj -> LN2 per 512-token half, pipelined across
             PE (stats matmuls, proj) / DVE (elementwise) / ACT (affine)

The softmax denominator is a partition-dim reduction done as a bf16 pairwise
tree on DVE (frees the PE of ~65k ones-matmul columns); LayerNorm stats stay
as all-ones stationary matmuls (cheap, and they broadcast for free).

The host transposes X/Y on the way in and the output on the way out, converts
everything the matmuls touch to bf16, and turns the bool mask into an
additive f32 bias (0 / -1e4) consumed by the exp activation.
"""

import math
import numpy as np
from contextlib import ExitStack

import ml_dtypes

import concourse.bass as bass
import concourse.mybir as mybir
import concourse.tile as tile
from concourse import bacc
from concourse.bass_utils import run_bass_kernel_spmd

P = 128
NX = 1024
NY = 1024
DIM = 1024
H = 8
KO = DIM // P          # 8 partition sub-tiles of the model dim
QC = 512               # moving-operand chunk
NQC = NX // QC         # 2
F32 = mybir.dt.float32
BF16 = mybir.dt.bfloat16
AF = mybir.ActivationFunctionType
ALU = mybir.AluOpType
SCALE = 1.0 / 32.0     # 1/sqrt(DIM)
EPS = 1e-5
DEBUG = False          # adds intermediate-tensor DRAM dumps (debugging only)


def _build():
    nc = bacc.Bacc("TRN2", target_bir_lowering=False, debug=False,
                   enable_asserts=False)

    # ---- DRAM I/O (per-core shapes) ----
    XT = nc.dram_tensor("XT", [DIM, NX], BF16, kind="ExternalInput").ap()
    YT = nc.dram_tensor("YT", [DIM, NY], BF16, kind="ExternalInput").ap()
    MB = nc.dram_tensor("MB", [NY], F32, kind="ExternalInput").ap()
    Wd = {}
    for w in ("Wq", "Wk", "Wv", "Wo"):
        Wd[w] = nc.dram_tensor(w, [DIM, DIM], BF16, kind="ExternalInput").ap()
    Vecs = {}
    for vname in ("bq", "bk", "bv", "bo", "g1", "b1", "g2", "b2"):
        Vecs[vname] = nc.dram_tensor(vname, [DIM], F32, kind="ExternalInput").ap()
    OT = nc.dram_tensor("OT", [DIM, NX], BF16, kind="ExternalOutput").ap()

    xt3 = XT.rearrange("(ko p) q -> p ko q", p=P)
    yt3 = YT.rearrange("(ko p) q -> p ko q", p=P)
    wq3 = Wd["Wq"].rearrange("(ko p) d -> p ko d", p=P)
    wk3 = Wd["Wk"].rearrange("(ko p) d -> p ko d", p=P)
    wv3 = Wd["Wv"].rearrange("(ko p) d -> p ko d", p=P)
    wo3 = Wd["Wo"].rearrange("(ko p) d -> p ko d", p=P)
    ot3 = OT.rearrange("(do p) q -> p do q", p=P)

    dbg = {}
    if DEBUG:
        for nm, shp, dt in [("d_qt", [P, KO, NX], BF16),
                            ("d_ktm0", [P, NY], BF16),
                            ("d_vm", [P, KO, DIM], BF16),
                            ("d_et00", [P, NX], BF16),
                            ("d_rc0", [P, NX], F32),
                            ("d_zt", [P, KO, NX], BF16),
                            ("d_o1t", [P, KO, NX], BF16),
                            ("d_z2t", [P, KO, NX], BF16)]:
            dbg[nm] = nc.dram_tensor(nm, shp, dt, kind="ExternalOutput").ap()

    with tile.TileContext(nc) as tc:
        with ExitStack() as octx:
            const = octx.enter_context(tc.tile_pool(name="const", bufs=1))
            persist = octx.enter_context(tc.tile_pool(name="persist", bufs=1))
            actp = octx.enter_context(tc.tile_pool(name="act", bufs=3))

            # ---- constants (issue the small DMAs first on the sync queue) ----
            ones_bf = const.tile([P, P], BF16, tag="onesbf", name="ones_bf")
            nc.vector.memset(ones_bf, 1.0)
            eps_sb = const.tile([P, 1], F32, tag="eps", name="eps_sb")
            nc.vector.memset(eps_sb, EPS)

            # vector constants: tiles now, DMAs issued after the big input
            # tensors (nothing reads them before ~45us)
            def vec_tile(name):
                return const.tile([P, KO], F32, tag=f"v_{name}", name=f"{name}_sb")

            vec_names = ("bq", "bk", "bv", "bo", "g1", "b1", "g2", "b2")
            vec_sb = {n: vec_tile(n) for n in vec_names}
            mb_sb = const.tile([P, KO], F32, tag="v_mb", name="mb_sb")
            bq_sb, bk_sb, bv_sb, bo_sb = (vec_sb[n] for n in vec_names[:4])
            g1_sb, b1_sb, g2_sb, b2_sb = (vec_sb[n] for n in vec_names[4:])

            def issue_vec_dmas():
                nc.sync.dma_start(mb_sb, MB.rearrange("(ko p) -> p ko", p=P))
                for n in vec_names:
                    nc.sync.dma_start(
                        vec_sb[n], Vecs[n].rearrange("(ko p) -> p ko", p=P))

            # ---- persistent tensors ----
            yt = persist.tile([P, KO, NY], BF16, tag="yt", name="yt")
            vm = persist.tile([P, KO, DIM], BF16, tag="vm", name="vm")
            wo_sb = persist.tile([P, KO, DIM], BF16, tag="wo", name="wo_sb")
            wkp = octx.enter_context(tc.tile_pool(name="wkp", bufs=2))
            ktmp = octx.enter_context(tc.tile_pool(name="ktmp", bufs=2))

            # big feature-major activation tiles (rotate through 3 slots)
            qt = actp.tile([P, KO, NX], BF16, tag="big", name="qt")

            # ================= Phase 1: V and Q projections =================
            with tc.tile_pool(name="io", bufs=1) as iop, \
                 tc.tile_pool(name="gp1", bufs=8, space="PSUM") as pp:
                wv_sb = iop.tile([P, KO, DIM], BF16, tag="wv", name="wv_sb")
                xt = iop.tile([P, KO, NX], BF16, tag="xt", name="xt")
                wq_sb = iop.tile([P, KO, DIM], BF16, tag="wq", name="wq_sb")
                # per-k chunked DMAs, interleaved by priority so the V proj
                # can start as soon as the first (yt, wv) chunk pair lands
                for k in range(KO):
                    nc.sync.dma_start(yt[:, k, :], yt3[:, k, :])
                    nc.sync.dma_start(wv_sb[:, k, :], wv3[:, k, :])
                for k in range(KO):
                    nc.sync.dma_start(xt[:, k, :], xt3[:, k, :])
                    nc.sync.dma_start(wq_sb[:, k, :], wq3[:, k, :])
                wk_tiles = []
                for h in range(2):
                    wkt = wkp.tile([P, KO, P], BF16, tag="wk", name=f"wk{h}")
                    nc.sync.dma_start(wkt, wk3[:, :, h * P:(h + 1) * P])
                    wk_tiles.append(wkt)
                issue_vec_dmas()
                nc.sync.dma_start(wo_sb, wo3)

                # V in natural (token-major) layout: V[y, n] = sum_k Y[y,k] Wv[k,n]
                # (bias bv is NOT added here: softmax rows sum to 1, so it is
                # folded into the attention epilogue instead)
                for yo in range(KO):
                    pss = [pp.tile([P, QC], F32, tag="ps", name=f"ps_v{yo}{ng}")
                           for ng in range(2)]
                    for k in range(KO):
                        for ng in range(2):
                            ns = slice(ng * QC, (ng + 1) * QC)
                            nc.tensor.matmul(
                                pss[ng],
                                lhsT=yt[:, k, yo * P:(yo + 1) * P],
                                rhs=wv_sb[:, k, ns],
                                start=(k == 0), stop=(k == KO - 1))
                    for ng in range(2):
                        ns = slice(ng * QC, (ng + 1) * QC)
                        nc.scalar.copy(vm[:, yo, ns], pss[ng])

                # Q feature-major: qt[p, do, q] = sum_k Wq[k, d] xt[k, q] + bq
                for do in range(KO):
                    for qc in range(NQC):
                        qs = slice(qc * QC, (qc + 1) * QC)
                        ps = pp.tile([P, QC], F32, tag="ps", name=f"ps_q{do}{qc}")
                        for k in range(KO):
                            nc.tensor.matmul(
                                ps,
                                lhsT=wq_sb[:, k, do * P:(do + 1) * P],
                                rhs=xt[:, k, qs],
                                start=(k == 0), stop=(k == KO - 1))
                        nc.scalar.activation(
                            qt[:, do, qs], ps, AF.Identity,
                            bias=bq_sb[:, do:do + 1], scale=1.0)

            if DEBUG:
                nc.sync.dma_start(dbg["d_qt"], qt)
                nc.sync.dma_start(dbg["d_vm"], vm)

            # ================= Phase 2: K proj + attention (pipelined) ======
            zt = actp.tile([P, KO, NX], BF16, tag="big", name="zt")

            with tc.tile_pool(name="kq", bufs=2, space="PSUM") as kqp, \
                 tc.tile_pool(name="lgp", bufs=2, space="PSUM") as lgp, \
                 tc.tile_pool(name="avp", bufs=1, space="PSUM") as avp, \
                 tc.tile_pool(name="exp", bufs=17) as ep, \
                 tc.tile_pool(name="prs", bufs=7) as prp, \
                 tc.tile_pool(name="den", bufs=2) as dnp, \
                 tc.tile_pool(name="rcp", bufs=2) as rcp:

                def kproj(h):
                    # K slab h: ktm_h[p, y] = sum_k Wk[k, h*128+p] yt[k, y] + bk
                    ktm_h = ktmp.tile([P, NY], BF16, tag="ktm", name=f"ktm{h}")
                    for qc in range(NQC):
                        qs = slice(qc * QC, (qc + 1) * QC)
                        pk = kqp.tile([P, QC], F32, tag="pk", name=f"pk{h}{qc}")
                        for k in range(KO):
                            nc.tensor.matmul(
                                pk, lhsT=wk_tiles[h][:, k, :], rhs=yt[:, k, qs],
                                start=(k == 0), stop=(k == KO - 1))
                        nc.scalar.activation(
                            ktm_h[:, qs], pk, AF.Identity,
                            bias=bk_sb[:, h:h + 1], scale=1.0)
                    if h + 2 < H:
                        # prefetch the h+2 weight chunk; issued after this
                        # head's matmuls so the 2-deep pool rotation can't
                        # clobber a chunk that still has unissued readers
                        wkt = wkp.tile([P, KO, P], BF16, tag="wk", name=f"wk{h+2}")
                        nc.sync.dma_start(wkt, wk3[:, :, (h + 2) * P:(h + 3) * P])
                        wk_tiles.append(wkt)
                    return ktm_h

                def logits_exp(h, ktm_h):
                    # logitsT[k, q] = sum_d KT_h[d, k] QT_h[d, q]; exp with
                    # mask bias per key (partition) and 1/32 scale
                    et = [ep.tile([P, NX], BF16, tag="exp", name=f"et{h}_{k}")
                          for k in range(KO)]
                    for kt in range(KO):
                        pl = lgp.tile([P, NX], F32, tag="lg", name=f"pl{h}{kt}")
                        for qc in range(NQC):
                            qs = slice(qc * QC, (qc + 1) * QC)
                            nc.tensor.matmul(
                                pl[:, qs],
                                lhsT=ktm_h[:, kt * P:(kt + 1) * P],
                                rhs=qt[:, h, qs],
                                start=True, stop=True)
                        nc.scalar.activation(
                            et[kt], pl, AF.Exp,
                            bias=mb_sb[:, kt:kt + 1], scale=SCALE)
                    return et

                def denom_tree(h, et):
                    # softmax denominator, part 1 (DVE only): cross-tile
                    # pairwise tree (bf16 2x) collapses the 8 key-slab tiles
                    # into one; issued at iteration start to fill the DVE
                    # bubble while the PE runs this head's K projection
                    l1 = [prp.tile([P, NX], BF16, tag="pr", name=f"l1_{h}{i}")
                          for i in range(4)]
                    for i in range(4):
                        nc.vector.tensor_add(l1[i], et[2 * i], et[2 * i + 1])
                    l2 = [prp.tile([P, NX], BF16, tag="pr", name=f"l2_{h}{i}")
                          for i in range(2)]
                    for i in range(2):
                        nc.vector.tensor_add(l2[i], l1[2 * i], l1[2 * i + 1])
                    den = dnp.tile([P, NX], BF16, tag="dn", name=f"den{h}")
                    nc.vector.tensor_add(den, l2[0], l2[1])
                    return den

                def denom_finish(h, den):
                    # part 2: a small all-ones matmul does the in-slab
                    # 128-partition reduction (and broadcasts for free)
                    rc = rcp.tile([P, NX], F32, tag="rc", name=f"rc{h}")
                    for qc in range(NQC):
                        qs = slice(qc * QC, (qc + 1) * QC)
                        pr = kqp.tile([P, QC], F32, tag="pk", name=f"pr{h}{qc}")
                        nc.tensor.matmul(pr, lhsT=ones_bf, rhs=den[:, qs],
                                         start=True, stop=True)
                        nc.vector.reciprocal_approx_fast(rc[:, qs], pr)
                    return rc

                def av_epi(h, et, rc):
                    # attnT_h[d, q] = sum_k V[k, d_h] expT[k, q]; normalize by
                    # the softmax denominator, add bv (softmax rows sum to 1,
                    # so + bv after normalize == bias inside the V proj) and
                    # the Q residual
                    pa = avp.tile([P, NX], F32, tag="av", name=f"pa{h}")
                    for kt in range(KO):
                        for qc in range(NQC):
                            qs = slice(qc * QC, (qc + 1) * QC)
                            nc.tensor.matmul(
                                pa[:, qs],
                                lhsT=vm[:, kt, h * P:(h + 1) * P],
                                rhs=et[kt][:, qs],
                                start=(kt == 0), stop=(kt == KO - 1))
                    # last head: per-qc epilogue so LN1's first chunk can
                    # start a bit earlier
                    for qs in ([slice(0, QC), slice(QC, NX)]
                               if h == H - 1 else [slice(0, NX)]):
                        nc.vector.tensor_mul(zt[:, h, qs], pa[:, qs], rc[:, qs])
                        nc.vector.scalar_tensor_tensor(
                            zt[:, h, qs], zt[:, h, qs], bv_sb[:, h:h + 1],
                            qt[:, h, qs], op0=ALU.add, op1=ALU.add)

                # software pipeline: head h's K proj / logits / exp overlap
                # head h-1's denominator (DVE tree + tiny ones-matmul) and
                # AV (PE).  Issue order is chosen per engine queue: the DVE
                # tree first (ready at iteration start), the PE denominator
                # matmuls after the logits matmuls (so the in-order PE queue
                # never waits on the tree).
                prev = None
                den_prev = None
                for h in range(H):
                    if prev is not None:
                        den_prev = denom_tree(h - 1, prev)
                    ktm_h = kproj(h)
                    et = logits_exp(h, ktm_h)
                    if DEBUG and h == 0:
                        nc.sync.dma_start(dbg["d_ktm0"], ktm_h)
                        nc.sync.dma_start(dbg["d_et00"], et[0])
                    if prev is not None:
                        rc_prev = denom_finish(h - 1, den_prev)
                        if DEBUG and h == 1:
                            nc.sync.dma_start(dbg["d_rc0"], rc_prev)
                        av_epi(h - 1, prev, rc_prev)
                    prev = et
                den_prev = denom_tree(H - 1, prev)
                rc_prev = denom_finish(H - 1, den_prev)
                av_epi(H - 1, prev, rc_prev)
                if DEBUG:
                    nc.sync.dma_start(dbg["d_zt"], zt)

            # ================= Phase 3: LN1 -> O proj + LN2 =================
            # LayerNorm over the model dim (partition direction): stats via
            # all-ones stationary matmuls (free broadcast), elementwise on
            # DVE, gamma/beta affine on ACT.
            with tc.tile_pool(name="sqp", bufs=2) as sqp, \
                 tc.tile_pool(name="stp", bufs=2) as stp, \
                 tc.tile_pool(name="out", bufs=2) as outp, \
                 tc.tile_pool(name="spp", bufs=2, space="PSUM") as spp, \
                 tc.tile_pool(name="gp3", bufs=4, space="PSUM") as pp3:

                def layernorm_chunk(in_sb, lo, hi, g_sb, b_sb, emit_out, lbl):
                    cs = hi - lo
                    qs = slice(lo, hi)
                    pmu = spp.tile([P, QC], F32, tag="pmu", name=f"pmu{lbl}{lo}")
                    for do in range(KO):
                        nc.tensor.matmul(pmu[:, :cs], lhsT=ones_bf,
                                         rhs=in_sb[:, do, qs],
                                         start=(do == 0), stop=(do == KO - 1))
                    sq = sqp.tile([P, KO, QC], BF16, tag="sq", name=f"sq{lbl}{lo}")
                    nc.scalar.square(sq[:, :, :cs], in_sb[:, :, qs])
                    ps2 = spp.tile([P, QC], F32, tag="ps2", name=f"ps2{lbl}{lo}")
                    for do in range(KO):
                        nc.tensor.matmul(ps2[:, :cs], lhsT=ones_bf,
                                         rhs=sq[:, do, :cs],
                                         start=(do == 0), stop=(do == KO - 1))
                    mu = stp.tile([P, QC], F32, tag="mu", name=f"mu{lbl}{lo}")[:, :cs]
                    nc.vector.tensor_scalar_mul(mu, pmu[:, :cs], 1.0 / DIM)
                    msq = stp.tile([P, QC], F32, tag="msq", name=f"msq{lbl}{lo}")[:, :cs]
                    nc.vector.tensor_mul(msq, mu, mu)
                    sd = stp.tile([P, QC], F32, tag="sd", name=f"sd{lbl}{lo}")[:, :cs]
                    nc.vector.scalar_tensor_tensor(
                        sd, ps2[:, :cs], 1.0 / DIM, msq,
                        op0=ALU.mult, op1=ALU.subtract)
                    nc.scalar.activation(sd, sd, AF.Sqrt, bias=eps_sb, scale=1.0)
                    rsig = stp.tile([P, QC], F32, tag="rsig", name=f"rsig{lbl}{lo}")[:, :cs]
                    nc.vector.reciprocal_approx_fast(rsig, sd)
                    mub = stp.tile([P, QC], BF16, tag="mub", name=f"mub{lbl}{lo}")[:, :cs]
                    nc.vector.tensor_copy(mub, mu)
                    rsb = stp.tile([P, QC], BF16, tag="rsb", name=f"rsb{lbl}{lo}")[:, :cs]
                    nc.vector.tensor_copy(rsb, rsig)
                    # t = (x - mu) * rsig over the whole [128, 8, cs] block
                    t = sqp.tile([P, KO, QC], BF16, tag="t", name=f"t{lbl}{lo}")
                    mu_b = mub.unsqueeze(1).broadcast_to([P, KO, cs])
                    rs_b = rsb.unsqueeze(1).broadcast_to([P, KO, cs])
                    nc.vector.tensor_sub(t[:, :, :cs], in_sb[:, :, qs], mu_b)
                    nc.vector.tensor_mul(t[:, :, :cs], t[:, :, :cs], rs_b)
                    for do in range(KO):
                        # out = t * g + b  (per-partition affine, DVE TS 4x)
                        emit_out(do, qs, t[:, do, :cs],
                                 g_sb[:, do:do + 1], b_sb[:, do:do + 1])

                o1t = actp.tile([P, KO, NX], BF16, tag="big", name="o1t")

                def emit_o1(do, qs, t, g_col, b_col):
                    # on ACT: it is idle right here while DVE is the pacer
                    nc.scalar.activation(o1t[:, do, qs], t, AF.Identity,
                                         bias=b_col, scale=g_col)

                z2t = actp.tile([P, KO, NX], BF16, tag="big", name="z2t")

                def oproj_qc(qc):
                    # HT[n, q] = sum_d Wo[d, n] O1T[d, q]; z2 = o1+relu(H+bo)
                    # (relu into a staging tile so the residual add is one
                    # merged DVE op per 512-token half)
                    qs = slice(qc * QC, (qc + 1) * QC)
                    ht = outp.tile([P, KO, QC], BF16, tag="ht", name=f"ht{qc}")
                    for no in range(KO):
                        po = pp3.tile([P, QC], F32, tag="po", name=f"po{no}{qc}")
                        for k in range(KO):
                            nc.tensor.matmul(
                                po,
                                lhsT=wo_sb[:, k, no * P:(no + 1) * P],
                                rhs=o1t[:, k, qs],
                                start=(k == 0), stop=(k == KO - 1))
                        nc.scalar.activation(ht[:, no, :], po, AF.Relu,
                                             bias=bo_sb[:, no:no + 1], scale=1.0)
                        if no == 3:   # first-half residual add starts mid-relu
                            nc.vector.tensor_add(z2t[:, 0:4, qs], ht[:, 0:4, :],
                                                 o1t[:, 0:4, qs])
                    nc.vector.tensor_add(z2t[:, 4:KO, qs], ht[:, 4:KO, :],
                                         o1t[:, 4:KO, qs])

                def make_emit_o2(lo, hi):
                    # affine writes into one staging tile; a single DMA per
                    # chunk replaces 8 per-do DMAs (issue + sem overhead)
                    cs = hi - lo
                    ost = outp.tile([P, KO, QC], BF16, tag="ost",
                                    name=f"ost{lo}")

                    def emit(do, qs, t, g_col, b_col):
                        nc.vector.tensor_scalar(ost[:, do, :cs], t, g_col,
                                                b_col, op0=ALU.mult, op1=ALU.add)
                        if do == KO - 1:
                            nc.sync.dma_start(ot3[:, :, lo:hi], ost[:, :, :cs])
                    return emit

                for qc in range(NQC):
                    layernorm_chunk(zt, qc * QC, (qc + 1) * QC,
                                    g1_sb, b1_sb, emit_o1, "a")
                if DEBUG:
                    nc.sync.dma_start(dbg["d_o1t"], o1t)
                # interleave the second O-proj half behind LN2's first chunk
                # so LN2's stats matmuls don't queue behind all of O-proj on
                # the in-order PE queue; LN2's shrinking chunks keep the
                # serial drain at the very end short
                oproj_qc(0)
                layernorm_chunk(z2t, 0, 512, g2_sb, b2_sb,
                                make_emit_o2(0, 512), "b")
                oproj_qc(1)
                if DEBUG:
                    nc.sync.dma_start(dbg["d_z2t"], z2t)
                for lo, hi in ((512, 768), (768, 1024)):
                    layernorm_chunk(z2t, lo, hi, g2_sb, b2_sb,
                                    make_emit_o2(lo, hi), "b")

    nc.compile()
    return nc


_CACHE = {}


def _get_nc():
    if "nc" not in _CACHE:
        _CACHE["nc"] = _build()
    return _CACHE["nc"]


def make_in_maps(X, Y, mask, Wq, bq, Wk, bk, Wv, bv, Wo, bo, g1, b1, g2, b2):
    bf = lambda a: np.ascontiguousarray(
        np.asarray(a, dtype=np.float32).astype(ml_dtypes.bfloat16))
    f = lambda a: np.ascontiguousarray(np.asarray(a, dtype=np.float32))
    shared = {
        "Wq": bf(Wq), "Wk": bf(Wk), "Wv": bf(Wv), "Wo": bf(Wo),
        "bq": f(bq), "bk": f(bk), "bv": f(bv), "bo": f(bo),
        "g1": f(g1), "b1": f(b1), "g2": f(g2), "b2": f(b2),
    }
    X = np.asarray(X, dtype=np.float32)
    Y = np.asarray(Y, dtype=np.float32)
    mask = np.asarray(mask)
    in_maps = []
    for b in range(8):
        mb = np.where(mask[b], np.float32(-1e4), np.float32(0.0)).astype(np.float32)
        in_maps.append({
            "XT": bf(X[b].T),
            "YT": bf(Y[b].T),
            "MB": mb,
            **shared,
        })
    return in_maps


def kernel(X, Y, mask, Wq, bq, Wk, bk, Wv, bv, Wo, bo, g1, b1, g2, b2,
           _trace=False):
    nc = _get_nc()
    in_maps = make_in_maps(X, Y, mask, Wq, bq, Wk, bk, Wv, bv, Wo, bo,
                           g1, b1, g2, b2)
    res = run_bass_kernel_spmd(nc, in_maps, core_ids=list(range(8)),
                               trace=_trace)
    out = np.stack([np.asarray(res.results[b]["OT"]).astype(np.float32).T
                    for b in range(8)])
    out = np.ascontiguousarray(out)
    if _trace:
        return out, res
    return out
